# revision 6
# baseline (speedup 1.0000x reference)
"""Trainium2 Bass kernel: DGCNN forward (4-layer GCN + Conv1d readout) on 8 NeuronCores.

Math restructuring (verified vs reference to 2e-7):
  With A = D^-1/2 (Adj + I) D^-1/2 and Mk / ck derived from the (tiny) weights,
    out = A(x M1 + A(x M2 + A(x M3 + A(x M4)))) + 1 c0 + v1 c1 + v2 c2 + v3 c3
  where vk = A^k 1 (graph-only vectors).  Every aggregation pass is width-16.

Device strategy (graph-parallel over 8 cores):
  - Nodes are permuted: degree-sorted, dealt into 128-row blocks round-robin
    across cores, so each core owns 49 blocks (6272 rows) spanning the degree
    spectrum and same-index blocks across cores have near-equal max degree.
  - Per pass: messages are fetched with dma_gather (256B rows) from a DRAM
    table into an ELL-padded [128, slots, 64] tile (dst j of block b -> SBUF
    partition j), then segment-summed with one strided DVE reduce per block.
  - int16 gather indices limit a window to 32768 rows; the 50176-row table is
    covered by two overlapping windows ([0,32768) and [17408,50176)), and each
    dst's edge list is split between the windows (balanced using the overlap),
    padded with a dedicated all-zero table row.
  - After each pass every core computes its own rows of the next table
    (T = dinv*(x Mk + S), via PE matmuls) and an 8-core AllGather rebuilds the
    full table in DRAM.
  - dinv pre/post scaling is folded into the table rows, so no per-edge
    normalization multiplies are needed.
"""

import dataclasses
import numpy as np

import concourse.bass as bass
import concourse.bacc as bacc
import concourse.tile as tile
from concourse import mybir
from concourse.bass_utils import run_bass_kernel_spmd
from concourse.masks import make_identity

F32 = mybir.dt.float32
I16 = mybir.dt.int16
AF = mybir.ActivationFunctionType


@dataclasses.dataclass(frozen=True)
class Cfg:
    N: int = 50000          # real nodes
    F: int = 64             # features
    NCORES: int = 8
    P: int = 128
    NBLK: int = 49          # dst blocks per core
    NGRP: int = 7           # gather groups per pass

    @property
    def PER(self):
        return self.NBLK * self.P

    @property
    def NPAD(self):
        return self.NCORES * self.PER

    @property
    def WA_LEN(self):
        return min(32768, self.NPAD)

    @property
    def WB_OFF(self):
        return self.NPAD - self.WA_LEN


CFG = Cfg()

# results of the last device run (for test harness profiling)
LAST_RESULTS = None


# --------------------------------------------------------------------------
# host preprocessing
# --------------------------------------------------------------------------

def _host_prep(inputs, cfg: Cfg):
    x = np.asarray(inputs["x"], np.float32)
    ei = np.asarray(inputs["edge_index"]).astype(np.int64)
    W = [np.asarray(inputs[f"W{i}"], np.float64) for i in range(4)]
    b = [np.asarray(inputs[f"b{i}"], np.float64) for i in range(4)]
    conv_w = np.asarray(inputs["conv_w"], np.float64)
    conv_b = np.asarray(inputs["conv_b"], np.float64)

    n = x.shape[0]
    assert n == cfg.N and x.shape[1] == cfg.F
    P, PER, NPAD, NBLK, NC = cfg.P, cfg.PER, cfg.NPAD, cfg.NBLK, cfg.NCORES

    src = np.concatenate([ei[0], np.arange(n, dtype=np.int64)])
    dst = np.concatenate([ei[1], np.arange(n, dtype=np.int64)])
    deg = np.bincount(dst, minlength=n).astype(np.float64)
    dinv = 1.0 / np.sqrt(np.maximum(deg, 1.0))

    # ---- weight-derived small matrices ----
    Cw = [conv_w[:, 0:64], conv_w[:, 64:128], conv_w[:, 128:192], conv_w[:, 192:193]]
    M1 = W[0] @ Cw[0].T
    M2 = W[0] @ W[1] @ Cw[1].T
    M3 = W[0] @ W[1] @ W[2] @ Cw[2].T
    M4 = W[0] @ W[1] @ W[2] @ W[3] @ Cw[3].T
    c0 = b[0] @ Cw[0].T + b[1] @ Cw[1].T + b[2] @ Cw[2].T + b[3] @ Cw[3].T + conv_b
    c1 = (b[0] @ W[1]) @ Cw[1].T + (b[1] @ W[2]) @ Cw[2].T + (b[2] @ W[3]) @ Cw[3].T
    c2 = (b[0] @ W[1] @ W[2]) @ Cw[2].T + (b[1] @ W[2] @ W[3]) @ Cw[3].T
    c3 = (b[0] @ W[1] @ W[2] @ W[3]) @ Cw[3].T

    def aggv(v):
        o = np.zeros(n)
        np.add.at(o, dst, (v * dinv)[src])
        return o * dinv

    v1 = aggv(np.ones(n))
    v2 = aggv(v1)
    v3 = aggv(v2)
    bias = (np.outer(np.ones(n), c0) + np.outer(v1, c1)
            + np.outer(v2, c2) + np.outer(v3, c3))  # [n, 16]

    # ---- permutation: degree sort, deal blocks round-robin across cores ----
    order = np.argsort(-deg, kind="stable")
    order_p = np.concatenate([order, np.full(NPAD - n, -1, np.int64)])
    assert NPAD - n >= 2
    # force a dummy (all-zero row) into window A: swap into (core 0, last block, j=P-1)
    rA = ((NBLK - 1) * NC + 0) * P + (P - 1)
    order_p[rA], order_p[NPAD - 1] = order_p[NPAD - 1], order_p[rA]

    g = np.arange(NPAD) // P
    j = np.arange(NPAD) % P
    npos_of_rank = (g % NC) * PER + (g // NC) * P + j
    pos2old = np.full(NPAD, -1, np.int64)
    pos2old[npos_of_rank] = order_p
    old2new = np.full(n, -1, np.int64)
    rmask = pos2old >= 0
    old2new[pos2old[rmask]] = np.nonzero(rmask)[0]

    zA = (NBLK - 1) * P + (P - 1)      # abs position of zero row in window A
    zB = NPAD - 2                      # abs position of zero row in window B
    assert pos2old[zA] < 0 and pos2old[zB] < 0
    assert zA < cfg.WA_LEN and zB >= cfg.WB_OFF

    # ---- per-edge window split, balanced via the overlap region ----
    s_new = old2new[src]
    d_new = old2new[dst]
    eo = np.argsort(d_new, kind="stable")
    s_s = s_new[eo]
    d_s = d_new[eo]
    E = len(s_s)
    starts = np.searchsorted(d_s, np.arange(NPAD + 1))

    isA = s_s < cfg.WB_OFF
    isB = s_s >= cfg.WA_LEN
    isF = ~(isA | isB)
    nAo = np.bincount(d_s, weights=isA, minlength=NPAD).astype(np.int64)
    nBo = np.bincount(d_s, weights=isB, minlength=NPAD).astype(np.int64)
    nf = np.bincount(d_s, weights=isF, minlength=NPAD).astype(np.int64)
    tot = nAo + nBo + nf
    half = (tot + 1) // 2
    nA = np.clip(half, nAo, nAo + nf)

    cFex = np.concatenate([[0], np.cumsum(isF)])
    frank = cFex[:-1] - cFex[starts[d_s]]          # rank among flex edges of this dst
    goA = isA | (isF & (frank < (nA - nAo)[d_s]))
    goB = ~goA
    cAex = np.concatenate([[0], np.cumsum(goA)])
    slotA = cAex[:-1] - cAex[starts[d_s]]
    cBex = np.concatenate([[0], np.cumsum(goB)])
    slotB = cBex[:-1] - cBex[starts[d_s]]

    blk_pos = (np.arange(NPAD) % PER) // P         # block id of each position
    nB = tot - nA
    SA = np.zeros(NBLK, np.int64)
    np.maximum.at(SA, blk_pos, nA)
    SB = np.zeros(NBLK, np.int64)
    np.maximum.at(SB, blk_pos, nB)
    SA = np.maximum(SA, 1)

    # ---- group blocks (balanced total slots) ----
    Stot = SA + SB
    per_grp = NBLK // cfg.NGRP + (1 if NBLK % cfg.NGRP else 0)
    groups = [[] for _ in range(cfg.NGRP)]
    gsum = np.zeros(cfg.NGRP)
    for bq in np.argsort(-Stot, kind="stable"):
        cand = sorted(range(cfg.NGRP),
                      key=lambda q: (len(groups[q]) >= per_grp, gsum[q], q))
        q = cand[0]
        groups[q].append(int(bq))
        gsum[q] += Stot[bq]

    oa = np.zeros(NBLK, np.int64)
    ob = np.zeros(NBLK, np.int64)
    grp_of = np.zeros(NBLK, np.int64)
    SAg = np.zeros(cfg.NGRP, np.int64)
    SBg = np.zeros(cfg.NGRP, np.int64)
    for q, bl in enumerate(groups):
        offa = 0
        for bq in bl:
            oa[bq] = offa
            offa += SA[bq]
            grp_of[bq] = q
        offb = 0
        for bq in bl:
            ob[bq] = offb
            offb += SB[bq]
        SAg[q] = offa
        SBg[q] = offb

    colA0 = np.zeros(cfg.NGRP, np.int64)
    colB0 = np.zeros(cfg.NGRP, np.int64)
    cur = 0
    for q in range(cfg.NGRP):
        colA0[q] = cur
        cur += SAg[q] * P // 16
        colB0[q] = cur
        cur += SBg[q] * P // 16
    idxcols = int(cur)

    # ---- build per-core idx tensors ----
    zA_rel = np.int16(zA)
    zB_rel = np.int16(zB - cfg.WB_OFF)
    idx_np = np.empty((NC, 128, idxcols), np.int16)
    # defaults: zero-row padding everywhere
    for q in range(cfg.NGRP):
        idx_np[:, :, colA0[q]:colA0[q] + SAg[q] * P // 16] = zA_rel
        idx_np[:, :, colB0[q]:colB0[q] + SBg[q] * P // 16] = zB_rel

    e_core = d_s // PER
    e_blk = (d_s % PER) // P
    e_j = d_s % P
    e_q = grp_of[e_blk]
    # linear position within the group's gather + column in the idx tensor
    posA = (oa[e_blk] + slotA) * P + e_j
    colA = colA0[e_q] + posA // 16
    rowA = posA % 16
    posB = (ob[e_blk] + slotB) * P + e_j
    colB = colB0[e_q] + posB // 16
    rowB = posB % 16
    valA = s_s.astype(np.int16)                   # window A offset is 0
    valB = (s_s - cfg.WB_OFF).astype(np.int16)
    for k in range(NC):
        mk = e_core == k
        mA = mk & goA
        mB = mk & goB
        for r in range(8):
            idx_np[k, rowA[mA] + 16 * r, colA[mA]] = valA[mA]
            idx_np[k, rowB[mB] + 16 * r, colB[mB]] = valB[mB]

    # ---- dense per-core arrays ----
    x_perm = np.zeros((NPAD, cfg.F), np.float32)
    x_perm[rmask] = x[pos2old[rmask]]
    dinv_perm = np.ones(NPAD, np.float32)
    dinv_perm[rmask] = dinv[pos2old[rmask]].astype(np.float32)
    bias_perm = np.zeros((NPAD, 16), np.float32)
    bias_perm[rmask] = bias[pos2old[rmask]].astype(np.float32)

    dinv_rows = dinv_perm.reshape(P, NPAD // P).copy()
    xT = [np.ascontiguousarray(x_perm[k * PER:(k + 1) * PER].T) for k in range(NC)]
    dinv_blk = [np.ascontiguousarray(dinv_perm[k * PER:(k + 1) * PER].reshape(NBLK, P).T)
                for k in range(NC)]
    dinv2_blk = [d * d for d in dinv_blk]
    bias_blk = [np.ascontiguousarray(
        bias_perm[k * PER:(k + 1) * PER].reshape(NBLK, P, 16).transpose(1, 0, 2))
        for k in range(NC)]
    mmats = np.ascontiguousarray(np.concatenate([M3, M2, M1], axis=1).astype(np.float32))
    m4 = np.ascontiguousarray(M4.astype(np.float32))

    layout = dict(SA=SA, SB=SB, groups=groups, oa=oa, ob=ob, SAg=SAg, SBg=SBg,
                  colA0=colA0, colB0=colB0, idxcols=idxcols)
    in_maps = []
    for k in range(NC):
        in_maps.append(dict(
            xraw=x_perm,
            idx=np.ascontiguousarray(idx_np[k]),
            xT=xT[k],
            dinv_rows=dinv_rows,
            dinv_blk=dinv_blk[k],
            dinv2_blk=dinv2_blk[k],
            bias_blk=bias_blk[k],
            mmats=mmats,
            m4=m4,
        ))
    return in_maps, layout, old2new


# --------------------------------------------------------------------------
# device module
# --------------------------------------------------------------------------

def _build_module(cfg: Cfg, layout):
    P, PER, NPAD, NBLK, NC = cfg.P, cfg.PER, cfg.NPAD, cfg.NBLK, cfg.NCORES
    SA, SB = layout["SA"], layout["SB"]
    groups, oa, ob = layout["groups"], layout["oa"], layout["ob"]
    SAg, SBg = layout["SAg"], layout["SBg"]
    colA0, colB0 = layout["colA0"], layout["colB0"]
    idxcols = layout["idxcols"]
    NROW = NPAD // P          # rows per partition in (p c) layouts

    nc = bacc.Bacc("TRN2", target_bir_lowering=False, debug=False, num_devices=NC)

    xraw = nc.dram_tensor("xraw", [NPAD, cfg.F], F32, kind="ExternalInput").ap()
    idx = nc.dram_tensor("idx", [128, idxcols], I16, kind="ExternalInput").ap()
    xT = nc.dram_tensor("xT", [cfg.F, PER], F32, kind="ExternalInput").ap()
    dinv_rows = nc.dram_tensor("dinv_rows", [P, NROW], F32, kind="ExternalInput").ap()
    dinv_blk = nc.dram_tensor("dinv_blk", [P, NBLK], F32, kind="ExternalInput").ap()
    dinv2_blk = nc.dram_tensor("dinv2_blk", [P, NBLK], F32, kind="ExternalInput").ap()
    bias_blk = nc.dram_tensor("bias_blk", [P, NBLK, 16], F32, kind="ExternalInput").ap()
    mmats = nc.dram_tensor("mmats", [cfg.F, 48], F32, kind="ExternalInput").ap()
    m4 = nc.dram_tensor("m4", [cfg.F, 16], F32, kind="ExternalInput").ap()
    out = nc.dram_tensor("out", [P, NBLK, 16], F32, kind="ExternalOutput").ap()

    with tile.TileContext(nc) as tc:
        with (
            tc.tile_pool(name="const", bufs=1) as cp,
            tc.tile_pool(name="dram", bufs=1, space="DRAM") as dp,
        ):
            idx_sb = cp.tile([128, idxcols], I16)
            nc.sync.dma_start(idx_sb[:], idx)
            xT_sb = cp.tile([cfg.F, PER], F32)
            nc.sync.dma_start(xT_sb[:], xT)
            mm_sb = cp.tile([cfg.F, 48], F32)
            nc.sync.dma_start(mm_sb[:], mmats)
            m4_sb = cp.tile([cfg.F, 16], F32)
            nc.sync.dma_start(m4_sb[:], m4)
            dr_sb = cp.tile([P, NROW], F32)
            nc.sync.dma_start(dr_sb[:], dinv_rows)
            db_sb = cp.tile([P, NBLK], F32)
            nc.sync.dma_start(db_sb[:], dinv_blk)
            d2_sb = cp.tile([P, NBLK], F32)
            nc.sync.dma_start(d2_sb[:], dinv2_blk)
            bias_sb = cp.tile([P, NBLK, 16], F32)
            nc.sync.dma_start(bias_sb[:], bias_blk)
            ident = cp.tile([P, P], F32)
            make_identity(nc, ident[:])

            xtab = dp.tile([NPAD, cfg.F], F32)
            ttab = dp.tile([NPAD, cfg.F], F32)
            ccin = [dp.tile([PER, 16], F32, name=f"ccin{i}") for i in range(3)]
            ccout = [dp.tile([NPAD, 16], F32, addr_space="Shared", name=f"ccout{i}") for i in range(3)]

            # ---- prologue: xtab = dinv * xraw ----
            xr_r = xraw.rearrange("(p c) f -> p c f", p=P)
            xt_r = xtab[:].rearrange("(p c) f -> p c f", p=P)
            nch = 4
            cc = NROW // nch
            assert cc * nch == NROW
            with tc.tile_pool(name="prol", bufs=2) as pp:
                for i in range(nch):
                    t = pp.tile([P, cc, cfg.F], F32, tag="prol")
                    nc.sync.dma_start(t[:], xr_r[:, i * cc:(i + 1) * cc, :])
                    nc.vector.tensor_tensor(
                        out=t[:], in0=t[:],
                        in1=dr_sb[:, i * cc:(i + 1) * cc].to_broadcast([P, cc, cfg.F]),
                        op=mybir.AluOpType.mult,
                    )
                    nc.sync.dma_start(xt_r[:, i * cc:(i + 1) * cc, :], t[:])

            with (
                tc.tile_pool(name="gath", bufs=2) as gp,
                tc.tile_pool(name="work", bufs=3) as wp,
                tc.tile_pool(name="stage", bufs=2) as sp,
                tc.tile_pool(name="psum", bufs=2, space="PSUM") as psp,
            ):
                def run_pass(tab, width, epi):
                    winA = tab[0:cfg.WA_LEN, :]
                    winB = tab[cfg.WB_OFF:NPAD, :]
                    for q, bl in enumerate(groups):
                        sag, sbg = int(SAg[q]), int(SBg[q])
                        s_all = sag + sbg
                        gt = gp.tile([P, s_all, cfg.F], F32, tag="gt")
                        nc.gpsimd.dma_gather(
                            out_ap=gt[:, 0:sag, :],
                            in_ap=winA,
                            idxs_ap=idx_sb[:, int(colA0[q]):int(colA0[q]) + sag * P // 16],
                            num_idxs=sag * P,
                            num_idxs_reg=sag * P,
                            elem_size=cfg.F,
                            single_packet=False,
                        )
                        if sbg:
                            nc.gpsimd.dma_gather(
                                out_ap=gt[:, sag:s_all, :],
                                in_ap=winB,
                                idxs_ap=idx_sb[:, int(colB0[q]):int(colB0[q]) + sbg * P // 16],
                                num_idxs=sbg * P,
                                num_idxs_reg=sbg * P,
                                elem_size=cfg.F,
                                single_packet=False,
                            )
                        for bq in bl:
                            a0, a1 = int(oa[bq]), int(oa[bq] + SA[bq])
                            acc = wp.tile([P, cfg.F], F32, tag="acc")
                            nc.vector.reduce_sum(
                                out=acc[:, 0:width],
                                in_=gt[:, a0:a1, 0:width].rearrange("p s f -> p f s"),
                                axis=mybir.AxisListType.X,
                            )
                            if SB[bq]:
                                b0_, b1_ = sag + int(ob[bq]), sag + int(ob[bq] + SB[bq])
                                acc2 = wp.tile([P, cfg.F], F32, tag="acc2")
                                nc.vector.reduce_sum(
                                    out=acc2[:, 0:width],
                                    in_=gt[:, b0_:b1_, 0:width].rearrange("p s f -> p f s"),
                                    axis=mybir.AxisListType.X,
                                )
                                nc.vector.tensor_add(
                                    out=acc[:, 0:width], in0=acc[:, 0:width],
                                    in1=acc2[:, 0:width])
                            epi(bq, acc)

                # ---- pass 4: gather x-table, build T3 ----
                st = sp.tile([P, NBLK, 16], F32, tag="stage")

                def epi4(bq, R):
                    rs = wp.tile([P, cfg.F], F32, tag="rs")
                    nc.scalar.activation(rs[:], R[:], AF.Copy, scale=db_sb[:, bq:bq + 1])
                    pT = psp.tile([cfg.F, P], F32, tag="pT")
                    nc.tensor.transpose(pT[:], rs[:], ident[:])
                    rsT = wp.tile([cfg.F, P], F32, tag="rsT")
                    nc.vector.tensor_copy(rsT[:], pT[:])
                    ps = psp.tile([P, 16], F32, tag="ps")
                    nc.tensor.matmul(out=ps[:], lhsT=xT_sb[:, bq * P:(bq + 1) * P],
                                     rhs=mm_sb[:, 0:16], start=True, stop=False)
                    nc.tensor.matmul(out=ps[:], lhsT=rsT[:], rhs=m4_sb[:],
                                     start=False, stop=True)
                    nc.scalar.activation(st[:, bq, :], ps[:], AF.Copy,
                                         scale=db_sb[:, bq:bq + 1])

                run_pass(xtab[:], cfg.F, epi4)
                nc.sync.dma_start(ccin[0][:].rearrange("(b p) f -> p b f", p=P), st[:])
                nc.gpsimd.collective_compute(
                    "AllGather", mybir.AluOpType.bypass,
                    replica_groups=[list(range(NC))],
                    ins=[ccin[0][:]], outs=[ccout[0][:]],
                )

                def restride(cco):
                    cr = cco[:].rearrange("(p c) f -> p c f", p=P)
                    tr = ttab[:].rearrange("(p c) f -> p c f", p=P)
                    for i in range(nch):
                        t = wp.tile([P, cc, 16], F32, tag="restride")
                        nc.sync.dma_start(t[:], cr[:, i * cc:(i + 1) * cc, :])
                        nc.sync.dma_start(tr[:, i * cc:(i + 1) * cc, 0:16], t[:])

                restride(ccout[0])

                # ---- passes 3 and 2: gather T, build next T ----
                def mk_epi(mcol, st_tile):
                    def epi(bq, R):
                        ps = psp.tile([P, 16], F32, tag="ps")
                        nc.tensor.matmul(out=ps[:],
                                         lhsT=xT_sb[:, bq * P:(bq + 1) * P],
                                         rhs=mm_sb[:, mcol:mcol + 16],
                                         start=True, stop=True)
                        ta = wp.tile([P, 16], F32, tag="ta")
                        nc.scalar.activation(ta[:], ps[:], AF.Copy,
                                             scale=db_sb[:, bq:bq + 1])
                        tb = wp.tile([P, 16], F32, tag="tb")
                        nc.scalar.activation(tb[:], R[:, 0:16], AF.Copy,
                                             scale=d2_sb[:, bq:bq + 1])
                        nc.vector.tensor_add(out=st_tile[:, bq, :], in0=ta[:], in1=tb[:])
                    return epi

                st3 = sp.tile([P, NBLK, 16], F32, tag="stage")
                run_pass(ttab[:], 16, mk_epi(16, st3))
                nc.sync.dma_start(ccin[1][:].rearrange("(b p) f -> p b f", p=P), st3[:])
                nc.gpsimd.collective_compute(
                    "AllGather", mybir.AluOpType.bypass,
                    replica_groups=[list(range(NC))],
                    ins=[ccin[1][:]], outs=[ccout[1][:]],
                )
                restride(ccout[1])

                st2 = sp.tile([P, NBLK, 16], F32, tag="stage")
                run_pass(ttab[:], 16, mk_epi(32, st2))
                nc.sync.dma_start(ccin[2][:].rearrange("(b p) f -> p b f", p=P), st2[:])
                nc.gpsimd.collective_compute(
                    "AllGather", mybir.AluOpType.bypass,
                    replica_groups=[list(range(NC))],
                    ins=[ccin[2][:]], outs=[ccout[2][:]],
                )
                restride(ccout[2])

                # ---- pass 1: final output ----
                sto = sp.tile([P, NBLK, 16], F32, tag="stage")

                def epi1(bq, R):
                    t1 = wp.tile([P, 16], F32, tag="ta")
                    nc.scalar.activation(t1[:], R[:, 0:16], AF.Copy,
                                         scale=db_sb[:, bq:bq + 1])
                    nc.vector.tensor_add(out=sto[:, bq, :], in0=t1[:],
                                         in1=bias_sb[:, bq, :])

                run_pass(ttab[:], 16, epi1)
                nc.sync.dma_start(out, sto[:])

    return nc


# --------------------------------------------------------------------------
# entry point
# --------------------------------------------------------------------------

def _run(inputs, cfg: Cfg, runner=None, **run_kwargs):
    """runner(nc, in_maps) -> list[dict] allows sim injection for testing."""
    global LAST_RESULTS
    in_maps, layout, old2new = _host_prep(inputs, cfg)
    nc = _build_module(cfg, layout)
    nc.compile()
    if runner is None:
        res = run_bass_kernel_spmd(nc, in_maps, core_ids=list(range(cfg.NCORES)),
                                   **run_kwargs)
        LAST_RESULTS = res
        outs = res.results
    else:
        outs = runner(nc, in_maps)
    full = np.empty((cfg.NPAD, 16), np.float32)
    for k in range(cfg.NCORES):
        o = np.asarray(outs[k]["out"])  # [P, NBLK, 16]
        full[k * cfg.PER:(k + 1) * cfg.PER] = o.transpose(1, 0, 2).reshape(cfg.PER, 16)
    return full[old2new]


def kernel(**inputs) -> np.ndarray:
    return _run(inputs, CFG)


# revision 8
# speedup vs baseline: 2.4218x; 2.4218x over previous
"""Trainium2 Bass kernel: DGCNN forward (4-layer GCN + Conv1d readout) on 8 NeuronCores.

Math restructuring (verified vs reference to 2e-7):
  With A = D^-1/2 (Adj + I) D^-1/2 and Mk / ck derived from the (tiny) weights,
    out = A(x M1 + A(x M2 + A(x M3 + A(x M4)))) + 1 c0 + v1 c1 + v2 c2 + v3 c3
  where vk = A^k 1 (graph-only vectors).  Every aggregation pass is width-16.

Device strategy (graph-parallel over 8 cores):
  - Nodes are permuted: degree-sorted, dealt into 128-row blocks round-robin
    across cores, so each core owns 49 blocks (6272 rows) spanning the degree
    spectrum and same-index blocks across cores have near-equal max degree.
  - Per pass: messages are fetched with dma_gather (256B rows) from a DRAM
    table into an ELL-padded [128, slots, 64] tile (dst j of block b -> SBUF
    partition j), then segment-summed with one strided DVE reduce per block.
  - int16 gather indices limit a window to 32768 rows; the 50176-row table is
    covered by two overlapping windows ([0,32768) and [17408,50176)), and each
    dst's edge list is split between the windows (balanced using the overlap),
    padded with a dedicated all-zero table row.
  - After each pass every core computes its own rows of the next table
    (T = dinv*(x Mk + S), via PE matmuls) and an 8-core AllGather rebuilds the
    full table in DRAM.
  - dinv pre/post scaling is folded into the table rows, so no per-edge
    normalization multiplies are needed.
"""

import dataclasses
import numpy as np

import concourse.bass as bass
import concourse.bacc as bacc
import concourse.tile as tile
from concourse import mybir
from concourse.bass_utils import run_bass_kernel_spmd
from concourse.masks import make_identity

F32 = mybir.dt.float32
I16 = mybir.dt.int16
AF = mybir.ActivationFunctionType


@dataclasses.dataclass(frozen=True)
class Cfg:
    N: int = 50000          # real nodes
    F: int = 64             # features
    NCORES: int = 8
    P: int = 128
    NBLK: int = 49          # dst blocks per core
    NGRP: int = 12          # gather groups per pass
    NQ: int = 4             # SWDGE queues (parallel Q7 desc-gen pairs)

    @property
    def PER(self):
        return self.NBLK * self.P

    @property
    def NPAD(self):
        return self.NCORES * self.PER

    @property
    def WA_LEN(self):
        return min(32768, self.NPAD)

    @property
    def WB_OFF(self):
        return self.NPAD - self.WA_LEN


CFG = Cfg()

# results of the last device run (for test harness profiling)
LAST_RESULTS = None


# --------------------------------------------------------------------------
# host preprocessing
# --------------------------------------------------------------------------

def _host_prep(inputs, cfg: Cfg):
    x = np.asarray(inputs["x"], np.float32)
    ei = np.asarray(inputs["edge_index"]).astype(np.int64)
    W = [np.asarray(inputs[f"W{i}"], np.float64) for i in range(4)]
    b = [np.asarray(inputs[f"b{i}"], np.float64) for i in range(4)]
    conv_w = np.asarray(inputs["conv_w"], np.float64)
    conv_b = np.asarray(inputs["conv_b"], np.float64)

    n = x.shape[0]
    assert n == cfg.N and x.shape[1] == cfg.F
    P, PER, NPAD, NBLK, NC = cfg.P, cfg.PER, cfg.NPAD, cfg.NBLK, cfg.NCORES

    src = np.concatenate([ei[0], np.arange(n, dtype=np.int64)])
    dst = np.concatenate([ei[1], np.arange(n, dtype=np.int64)])
    deg = np.bincount(dst, minlength=n).astype(np.float64)
    dinv = 1.0 / np.sqrt(np.maximum(deg, 1.0))

    # ---- weight-derived small matrices ----
    Cw = [conv_w[:, 0:64], conv_w[:, 64:128], conv_w[:, 128:192], conv_w[:, 192:193]]
    M1 = W[0] @ Cw[0].T
    M2 = W[0] @ W[1] @ Cw[1].T
    M3 = W[0] @ W[1] @ W[2] @ Cw[2].T
    M4 = W[0] @ W[1] @ W[2] @ W[3] @ Cw[3].T
    c0 = b[0] @ Cw[0].T + b[1] @ Cw[1].T + b[2] @ Cw[2].T + b[3] @ Cw[3].T + conv_b
    c1 = (b[0] @ W[1]) @ Cw[1].T + (b[1] @ W[2]) @ Cw[2].T + (b[2] @ W[3]) @ Cw[3].T
    c2 = (b[0] @ W[1] @ W[2]) @ Cw[2].T + (b[1] @ W[2] @ W[3]) @ Cw[3].T
    c3 = (b[0] @ W[1] @ W[2] @ W[3]) @ Cw[3].T

    def aggv(v):
        o = np.zeros(n)
        np.add.at(o, dst, (v * dinv)[src])
        return o * dinv

    v1 = aggv(np.ones(n))
    v2 = aggv(v1)
    v3 = aggv(v2)
    bias = (np.outer(np.ones(n), c0) + np.outer(v1, c1)
            + np.outer(v2, c2) + np.outer(v3, c3))  # [n, 16]

    # ---- permutation: degree sort, deal blocks round-robin across cores ----
    order = np.argsort(-deg, kind="stable")
    order_p = np.concatenate([order, np.full(NPAD - n, -1, np.int64)])
    assert NPAD - n >= 2
    # force a dummy (all-zero row) into window A: swap into (core 0, last block, j=P-1)
    rA = ((NBLK - 1) * NC + 0) * P + (P - 1)
    order_p[rA], order_p[NPAD - 1] = order_p[NPAD - 1], order_p[rA]

    g = np.arange(NPAD) // P
    j = np.arange(NPAD) % P
    npos_of_rank = (g % NC) * PER + (g // NC) * P + j
    pos2old = np.full(NPAD, -1, np.int64)
    pos2old[npos_of_rank] = order_p
    old2new = np.full(n, -1, np.int64)
    rmask = pos2old >= 0
    old2new[pos2old[rmask]] = np.nonzero(rmask)[0]

    zA = (NBLK - 1) * P + (P - 1)      # abs position of zero row in window A
    zB = NPAD - 2                      # abs position of zero row in window B
    assert pos2old[zA] < 0 and pos2old[zB] < 0
    assert zA < cfg.WA_LEN and zB >= cfg.WB_OFF

    # ---- per-edge window split, balanced via the overlap region ----
    s_new = old2new[src]
    d_new = old2new[dst]
    eo = np.argsort(d_new, kind="stable")
    s_s = s_new[eo]
    d_s = d_new[eo]
    E = len(s_s)
    starts = np.searchsorted(d_s, np.arange(NPAD + 1))

    isA = s_s < cfg.WB_OFF
    isB = s_s >= cfg.WA_LEN
    isF = ~(isA | isB)
    nAo = np.bincount(d_s, weights=isA, minlength=NPAD).astype(np.int64)
    nBo = np.bincount(d_s, weights=isB, minlength=NPAD).astype(np.int64)
    nf = np.bincount(d_s, weights=isF, minlength=NPAD).astype(np.int64)
    tot = nAo + nBo + nf

    # per-block (over all cores) minimal slot budget: SA+SB = max(max tot, max nAo + max nBo)
    blk_pos = (np.arange(NPAD) % PER) // P
    mT = np.zeros(NBLK, np.int64); np.maximum.at(mT, blk_pos, tot)
    mA = np.zeros(NBLK, np.int64); np.maximum.at(mA, blk_pos, nAo)
    mB = np.zeros(NBLK, np.int64); np.maximum.at(mB, blk_pos, nBo)
    M = np.maximum(mT, mA + mB)
    SA = np.clip((M + 1) // 2, mA, M - mB)
    SA = np.maximum(SA, 1)
    SB = M - SA
    SAp = SA[blk_pos]               # per-position block budgets
    SBp = SB[blk_pos]
    nA = np.clip(tot - SBp, nAo, np.minimum(nAo + nf, SAp))

    cFex = np.concatenate([[0], np.cumsum(isF)])
    frank = cFex[:-1] - cFex[starts[d_s]]          # rank among flex edges of this dst
    goA = isA | (isF & (frank < (nA - nAo)[d_s]))
    goB = ~goA
    cAex = np.concatenate([[0], np.cumsum(goA)])
    slotA = cAex[:-1] - cAex[starts[d_s]]
    cBex = np.concatenate([[0], np.cumsum(goB)])
    slotB = cBex[:-1] - cBex[starts[d_s]]

    nB = tot - nA
    assert (nA <= SAp).all() and (nB <= SBp).all()

    # ---- group blocks (balanced total slots) ----
    Stot = SA + SB
    per_grp = NBLK // cfg.NGRP + (1 if NBLK % cfg.NGRP else 0)
    groups = [[] for _ in range(cfg.NGRP)]
    gsum = np.zeros(cfg.NGRP)
    for bq in np.argsort(-Stot, kind="stable"):
        cand = sorted(range(cfg.NGRP),
                      key=lambda q: (len(groups[q]) >= per_grp, gsum[q], q))
        q = cand[0]
        groups[q].append(int(bq))
        gsum[q] += Stot[bq]

    oa = np.zeros(NBLK, np.int64)
    ob = np.zeros(NBLK, np.int64)
    grp_of = np.zeros(NBLK, np.int64)
    SAg = np.zeros(cfg.NGRP, np.int64)
    SBg = np.zeros(cfg.NGRP, np.int64)
    for q, bl in enumerate(groups):
        offa = 0
        for bq in bl:
            oa[bq] = offa
            offa += SA[bq]
            grp_of[bq] = q
        offb = 0
        for bq in bl:
            ob[bq] = offb
            offb += SB[bq]
        SAg[q] = offa
        SBg[q] = offb

    colA0 = np.zeros(cfg.NGRP, np.int64)
    colB0 = np.zeros(cfg.NGRP, np.int64)
    cur = 0
    for q in range(cfg.NGRP):
        colA0[q] = cur
        cur += SAg[q] * P // 16
        colB0[q] = cur
        cur += SBg[q] * P // 16
    idxcols = int(cur)

    # ---- build per-core idx tensors ----
    zA_rel = np.int16(zA)
    zB_rel = np.int16(zB - cfg.WB_OFF)
    idx_np = np.empty((NC, 128, idxcols), np.int16)
    # defaults: zero-row padding everywhere
    for q in range(cfg.NGRP):
        idx_np[:, :, colA0[q]:colA0[q] + SAg[q] * P // 16] = zA_rel
        idx_np[:, :, colB0[q]:colB0[q] + SBg[q] * P // 16] = zB_rel

    e_core = d_s // PER
    e_blk = (d_s % PER) // P
    e_j = d_s % P
    e_q = grp_of[e_blk]
    # linear position within the group's gather + column in the idx tensor
    posA = (oa[e_blk] + slotA) * P + e_j
    colA = colA0[e_q] + posA // 16
    rowA = posA % 16
    posB = (ob[e_blk] + slotB) * P + e_j
    colB = colB0[e_q] + posB // 16
    rowB = posB % 16
    valA = s_s.astype(np.int16)                   # window A offset is 0
    valB = (s_s - cfg.WB_OFF).astype(np.int16)
    for k in range(NC):
        mk = e_core == k
        mA = mk & goA
        mB = mk & goB
        for r in range(8):
            idx_np[k, rowA[mA] + 16 * r, colA[mA]] = valA[mA]
            idx_np[k, rowB[mB] + 16 * r, colB[mB]] = valB[mB]

    # ---- dense per-core arrays ----
    x_perm = np.zeros((NPAD, cfg.F), np.float32)
    x_perm[rmask] = x[pos2old[rmask]]
    dinv_perm = np.ones(NPAD, np.float32)
    dinv_perm[rmask] = dinv[pos2old[rmask]].astype(np.float32)
    bias_perm = np.zeros((NPAD, 16), np.float32)
    bias_perm[rmask] = bias[pos2old[rmask]].astype(np.float32)

    dinv_rows = dinv_perm.reshape(P, NPAD // P).copy()
    xT = [np.ascontiguousarray(x_perm[k * PER:(k + 1) * PER].T) for k in range(NC)]
    dinv_blk = [np.ascontiguousarray(dinv_perm[k * PER:(k + 1) * PER].reshape(NBLK, P).T)
                for k in range(NC)]
    dinv2_blk = [d * d for d in dinv_blk]
    bias_blk = [np.ascontiguousarray(
        bias_perm[k * PER:(k + 1) * PER].reshape(NBLK, P, 16).transpose(1, 0, 2))
        for k in range(NC)]
    mmats = np.ascontiguousarray(np.concatenate([M3, M2, M1], axis=1).astype(np.float32))
    m4 = np.ascontiguousarray(M4.astype(np.float32))

    layout = dict(SA=SA, SB=SB, groups=groups, oa=oa, ob=ob, SAg=SAg, SBg=SBg,
                  colA0=colA0, colB0=colB0, idxcols=idxcols)
    in_maps = []
    for k in range(NC):
        in_maps.append(dict(
            xraw=x_perm,
            idx=np.ascontiguousarray(idx_np[k]),
            xT=xT[k],
            dinv_rows=dinv_rows,
            dinv_blk=dinv_blk[k],
            dinv2_blk=dinv2_blk[k],
            bias_blk=bias_blk[k],
            mmats=mmats,
            m4=m4,
        ))
    return in_maps, layout, old2new


# --------------------------------------------------------------------------
# device module
# --------------------------------------------------------------------------

def _build_module(cfg: Cfg, layout):
    P, PER, NPAD, NBLK, NC = cfg.P, cfg.PER, cfg.NPAD, cfg.NBLK, cfg.NCORES
    SA, SB = layout["SA"], layout["SB"]
    groups, oa, ob = layout["groups"], layout["oa"], layout["ob"]
    SAg, SBg = layout["SAg"], layout["SBg"]
    colA0, colB0 = layout["colA0"], layout["colB0"]
    idxcols = layout["idxcols"]
    NROW = NPAD // P          # rows per partition in (p c) layouts

    nc = bacc.Bacc("TRN2", target_bir_lowering=False, debug=False, num_devices=NC,
                   num_swdge_queues=cfg.NQ)

    xraw = nc.dram_tensor("xraw", [NPAD, cfg.F], F32, kind="ExternalInput").ap()
    idx = nc.dram_tensor("idx", [128, idxcols], I16, kind="ExternalInput").ap()
    xT = nc.dram_tensor("xT", [cfg.F, PER], F32, kind="ExternalInput").ap()
    dinv_rows = nc.dram_tensor("dinv_rows", [P, NROW], F32, kind="ExternalInput").ap()
    dinv_blk = nc.dram_tensor("dinv_blk", [P, NBLK], F32, kind="ExternalInput").ap()
    dinv2_blk = nc.dram_tensor("dinv2_blk", [P, NBLK], F32, kind="ExternalInput").ap()
    bias_blk = nc.dram_tensor("bias_blk", [P, NBLK, 16], F32, kind="ExternalInput").ap()
    mmats = nc.dram_tensor("mmats", [cfg.F, 48], F32, kind="ExternalInput").ap()
    m4 = nc.dram_tensor("m4", [cfg.F, 16], F32, kind="ExternalInput").ap()
    out = nc.dram_tensor("out", [P, NBLK, 16], F32, kind="ExternalOutput").ap()

    with tile.TileContext(nc) as tc:
        with (
            tc.tile_pool(name="const", bufs=1) as cp,
            tc.tile_pool(name="dram", bufs=1, space="DRAM") as dp,
        ):
            idx_sb = cp.tile([128, idxcols], I16)
            nc.sync.dma_start(idx_sb[:], idx)
            xT_sb = cp.tile([cfg.F, PER], F32)
            nc.sync.dma_start(xT_sb[:], xT)
            mm_sb = cp.tile([cfg.F, 48], F32)
            nc.sync.dma_start(mm_sb[:], mmats)
            m4_sb = cp.tile([cfg.F, 16], F32)
            nc.sync.dma_start(m4_sb[:], m4)
            dr_sb = cp.tile([P, NROW], F32)
            nc.sync.dma_start(dr_sb[:], dinv_rows)
            db_sb = cp.tile([P, NBLK], F32)
            nc.sync.dma_start(db_sb[:], dinv_blk)
            d2_sb = cp.tile([P, NBLK], F32)
            nc.sync.dma_start(d2_sb[:], dinv2_blk)
            bias_sb = cp.tile([P, NBLK, 16], F32)
            nc.sync.dma_start(bias_sb[:], bias_blk)
            ident = cp.tile([P, P], F32)
            make_identity(nc, ident[:])

            xtab = dp.tile([NPAD, cfg.F], F32)
            ttab = dp.tile([NPAD, cfg.F], F32)
            ccin = [dp.tile([PER, 16], F32, name=f"ccin{i}") for i in range(3)]
            ccout = [dp.tile([NPAD, 16], F32, addr_space="Shared", name=f"ccout{i}") for i in range(3)]

            # ---- prologue: xtab = dinv * xraw ----
            xr_r = xraw.rearrange("(p c) f -> p c f", p=P)
            xt_r = xtab[:].rearrange("(p c) f -> p c f", p=P)
            nch = 4
            cc = NROW // nch
            assert cc * nch == NROW
            with tc.tile_pool(name="prol", bufs=2) as pp:
                for i in range(nch):
                    t = pp.tile([P, cc, cfg.F], F32, tag="prol")
                    nc.sync.dma_start(t[:], xr_r[:, i * cc:(i + 1) * cc, :])
                    nc.vector.tensor_tensor(
                        out=t[:], in0=t[:],
                        in1=dr_sb[:, i * cc:(i + 1) * cc].to_broadcast([P, cc, cfg.F]),
                        op=mybir.AluOpType.mult,
                    )
                    nc.sync.dma_start(xt_r[:, i * cc:(i + 1) * cc, :], t[:])

            with (
                tc.tile_pool(name="gath", bufs=4) as gp,
                tc.tile_pool(name="work", bufs=3) as wp,
                tc.tile_pool(name="stage", bufs=2) as sp,
                tc.tile_pool(name="psum", bufs=2, space="PSUM") as psp,
            ):
                qctr = [0]

                def next_q():
                    q = qctr[0] % cfg.NQ
                    qctr[0] += 1
                    return q

                def run_pass(tab, width, epi):
                    winA = tab[0:cfg.WA_LEN, :]
                    winB = tab[cfg.WB_OFF:NPAD, :]
                    for q, bl in enumerate(groups):
                        sag, sbg = int(SAg[q]), int(SBg[q])
                        s_all = sag + sbg
                        gt = gp.tile([P, s_all, cfg.F], F32, tag="gt")
                        nc.gpsimd.dma_gather(
                            out_ap=gt[:, 0:sag, :],
                            in_ap=winA,
                            idxs_ap=idx_sb[:, int(colA0[q]):int(colA0[q]) + sag * P // 16],
                            num_idxs=sag * P,
                            num_idxs_reg=sag * P,
                            elem_size=cfg.F,
                            single_packet=False,
                            queue_num=next_q(),
                        )
                        if sbg:
                            nc.gpsimd.dma_gather(
                                out_ap=gt[:, sag:s_all, :],
                                in_ap=winB,
                                idxs_ap=idx_sb[:, int(colB0[q]):int(colB0[q]) + sbg * P // 16],
                                num_idxs=sbg * P,
                                num_idxs_reg=sbg * P,
                                elem_size=cfg.F,
                                single_packet=False,
                                queue_num=next_q(),
                            )
                        for bq in bl:
                            a0, a1 = int(oa[bq]), int(oa[bq] + SA[bq])
                            acc = wp.tile([P, cfg.F], F32, tag="acc")
                            nc.vector.reduce_sum(
                                out=acc[:, 0:width],
                                in_=gt[:, a0:a1, 0:width].rearrange("p s f -> p f s"),
                                axis=mybir.AxisListType.X,
                            )
                            if SB[bq]:
                                b0_, b1_ = sag + int(ob[bq]), sag + int(ob[bq] + SB[bq])
                                acc2 = wp.tile([P, cfg.F], F32, tag="acc2")
                                nc.vector.reduce_sum(
                                    out=acc2[:, 0:width],
                                    in_=gt[:, b0_:b1_, 0:width].rearrange("p s f -> p f s"),
                                    axis=mybir.AxisListType.X,
                                )
                                nc.vector.tensor_add(
                                    out=acc[:, 0:width], in0=acc[:, 0:width],
                                    in1=acc2[:, 0:width])
                            epi(bq, acc)

                # ---- pass 4: gather x-table, build T3 ----
                st = sp.tile([P, NBLK, 16], F32, tag="stage")

                def epi4(bq, R):
                    rs = wp.tile([P, cfg.F], F32, tag="rs")
                    nc.scalar.activation(rs[:], R[:], AF.Copy, scale=db_sb[:, bq:bq + 1])
                    pT = psp.tile([cfg.F, P], F32, tag="pT")
                    nc.tensor.transpose(pT[:], rs[:], ident[:])
                    rsT = wp.tile([cfg.F, P], F32, tag="rsT")
                    nc.vector.tensor_copy(rsT[:], pT[:])
                    ps = psp.tile([P, 16], F32, tag="ps")
                    nc.tensor.matmul(out=ps[:], lhsT=xT_sb[:, bq * P:(bq + 1) * P],
                                     rhs=mm_sb[:, 0:16], start=True, stop=False)
                    nc.tensor.matmul(out=ps[:], lhsT=rsT[:], rhs=m4_sb[:],
                                     start=False, stop=True)
                    nc.scalar.activation(st[:, bq, :], ps[:], AF.Copy,
                                         scale=db_sb[:, bq:bq + 1])

                run_pass(xtab[:], cfg.F, epi4)
                nc.sync.dma_start(ccin[0][:].rearrange("(b p) f -> p b f", p=P), st[:])
                nc.gpsimd.collective_compute(
                    "AllGather", mybir.AluOpType.bypass,
                    replica_groups=[list(range(NC))],
                    ins=[ccin[0][:]], outs=[ccout[0][:]],
                )

                def restride(cco):
                    cr = cco[:].rearrange("(p c) f -> p c f", p=P)
                    tr = ttab[:].rearrange("(p c) f -> p c f", p=P)
                    for i in range(nch):
                        t = wp.tile([P, cc, 16], F32, tag="restride")
                        nc.sync.dma_start(t[:], cr[:, i * cc:(i + 1) * cc, :])
                        nc.sync.dma_start(tr[:, i * cc:(i + 1) * cc, 0:16], t[:])

                restride(ccout[0])

                # ---- passes 3 and 2: gather T, build next T ----
                def mk_epi(mcol, st_tile):
                    def epi(bq, R):
                        ps = psp.tile([P, 16], F32, tag="ps")
                        nc.tensor.matmul(out=ps[:],
                                         lhsT=xT_sb[:, bq * P:(bq + 1) * P],
                                         rhs=mm_sb[:, mcol:mcol + 16],
                                         start=True, stop=True)
                        ta = wp.tile([P, 16], F32, tag="ta")
                        nc.scalar.activation(ta[:], ps[:], AF.Copy,
                                             scale=db_sb[:, bq:bq + 1])
                        tb = wp.tile([P, 16], F32, tag="tb")
                        nc.scalar.activation(tb[:], R[:, 0:16], AF.Copy,
                                             scale=d2_sb[:, bq:bq + 1])
                        nc.vector.tensor_add(out=st_tile[:, bq, :], in0=ta[:], in1=tb[:])
                    return epi

                st3 = sp.tile([P, NBLK, 16], F32, tag="stage")
                run_pass(ttab[:], 16, mk_epi(16, st3))
                nc.sync.dma_start(ccin[1][:].rearrange("(b p) f -> p b f", p=P), st3[:])
                nc.gpsimd.collective_compute(
                    "AllGather", mybir.AluOpType.bypass,
                    replica_groups=[list(range(NC))],
                    ins=[ccin[1][:]], outs=[ccout[1][:]],
                )
                restride(ccout[1])

                st2 = sp.tile([P, NBLK, 16], F32, tag="stage")
                run_pass(ttab[:], 16, mk_epi(32, st2))
                nc.sync.dma_start(ccin[2][:].rearrange("(b p) f -> p b f", p=P), st2[:])
                nc.gpsimd.collective_compute(
                    "AllGather", mybir.AluOpType.bypass,
                    replica_groups=[list(range(NC))],
                    ins=[ccin[2][:]], outs=[ccout[2][:]],
                )
                restride(ccout[2])

                # ---- pass 1: final output ----
                sto = sp.tile([P, NBLK, 16], F32, tag="stage")

                def epi1(bq, R):
                    t1 = wp.tile([P, 16], F32, tag="ta")
                    nc.scalar.activation(t1[:], R[:, 0:16], AF.Copy,
                                         scale=db_sb[:, bq:bq + 1])
                    nc.vector.tensor_add(out=sto[:, bq, :], in0=t1[:],
                                         in1=bias_sb[:, bq, :])

                run_pass(ttab[:], 16, epi1)
                nc.sync.dma_start(out, sto[:])

    return nc


# --------------------------------------------------------------------------
# entry point
# --------------------------------------------------------------------------

def _run(inputs, cfg: Cfg, runner=None, **run_kwargs):
    """runner(nc, in_maps) -> list[dict] allows sim injection for testing."""
    global LAST_RESULTS
    in_maps, layout, old2new = _host_prep(inputs, cfg)
    nc = _build_module(cfg, layout)
    nc.compile()
    if runner is None:
        res = run_bass_kernel_spmd(nc, in_maps, core_ids=list(range(cfg.NCORES)),
                                   **run_kwargs)
        LAST_RESULTS = res
        outs = res.results
    else:
        outs = runner(nc, in_maps)
    full = np.empty((cfg.NPAD, 16), np.float32)
    for k in range(cfg.NCORES):
        o = np.asarray(outs[k]["out"])  # [P, NBLK, 16]
        full[k * cfg.PER:(k + 1) * cfg.PER] = o.transpose(1, 0, 2).reshape(cfg.PER, 16)
    return full[old2new]


def kernel(**inputs) -> np.ndarray:
    return _run(inputs, CFG)
